# revision 17
# baseline (speedup 1.0000x reference)
"""Trainium2 Bass kernel for batched cross-attention with gaussian guide mask.

Reference computation (per batch b):
  Q   = query @ Wq.T                      # [Tq, A]
  att = (Q @ K.T / sqrt(A)) * guide       # guide[n] = exp(-(step-(n+1)/N)^2/TEMP)
  att = where(mask, -inf, att)
  out = softmax(att, axis=-1) @ V         # [Tq, E]

Sharding: data-parallel over batch. Core b handles batch b (B == 8 == n_cores).

Device-side layout choices (host does layout-only prep: transposes/casts):
  qT  = query[b].T   [L, Tq]   - so Q^T comes out of PE with A on partitions
  kT  = K[b].T       [A, N]    - guide and 1/sqrt(A) folded in on device
  v   = V[b]         [N, E]    - natural; AV contraction tiles n on partitions
  wqT = Wq.T         [L, A]
  msk = mask[b] u8   [Tq, N]
Softmax is computed without max-subtraction (att values are O(5), exp is safe
in f32, and softmax is shift-invariant); masked lanes are set to -200 before
exp so exp underflows to 0 and the fused accumulate row-sum is exact.
Normalization is applied to the [128, 512] output tile instead of the
[128, 2048] score tile (linearity of the AV matmul).
"""

import math

import ml_dtypes
import numpy as np

import concourse.bass as bass
import concourse.mybir as mybir
import concourse.tile as tile
from concourse import bacc
from concourse.bass import ts
from concourse.bass_utils import run_bass_kernel_spmd
from concourse.masks import make_identity

B, TQ, N = 8, 1024, 2048
L, A, E = 1024, 128, 512
TEMP = 0.08
P = 128
LT = L // P    # 8 l-tiles (contraction tiles of the Q projection)
TT = TQ // P   # 8 t-tiles (rows of attention, 128 at a time)
NT = N // P    # 16 n-tiles (contraction tiles of the AV matmul)
NEG = -200.0   # masked logit value; exp(-200) underflows to exactly 0 in f32

import os

USE_DMA_TRANSPOSE = os.environ.get("KDMAT", "0") == "1"

F32 = mybir.dt.float32
F32R = mybir.dt.float32r
F16 = mybir.dt.float16
U8 = mybir.dt.uint8


def build_nc():
    nc = bacc.Bacc("TRN2", target_bir_lowering=False, debug=False, enable_asserts=False, num_devices=B)

    qT = nc.dram_tensor("qT", [L, TQ], F16, kind="ExternalInput").ap()
    kT = nc.dram_tensor("kT", [A, N], F16, kind="ExternalInput").ap()
    v = nc.dram_tensor("v", [N, E], F16, kind="ExternalInput").ap()
    wqT = nc.dram_tensor("wqT", [L, A], F16, kind="ExternalInput").ap()
    stp = nc.dram_tensor("stp", [1, 1], F32, kind="ExternalInput").ap()
    msk = nc.dram_tensor("msk", [TQ, N], U8, kind="ExternalInput").ap()
    out = nc.dram_tensor("out", [TQ, E], F32, kind="ExternalOutput").ap()

    with tile.TileContext(nc) as tc:
        with (
            tc.tile_pool(name="const", bufs=1) as const,
            tc.tile_pool(name="setup", bufs=1) as setup,
            tc.tile_pool(name="mpool", bufs=3) as mpool,
            tc.tile_pool(name="spool", bufs=3) as spool,
            tc.tile_pool(name="stpool", bufs=3) as stpool,
            tc.tile_pool(name="opool", bufs=3) as opool,
            tc.tile_pool(name="small", bufs=6) as small,
            tc.tile_pool(
                name="psA", bufs=3 if USE_DMA_TRANSPOSE else 2, space="PSUM"
            ) as psA,
            tc.tile_pool(name="psO", bufs=2, space="PSUM") as psO,
        ):
            # ---- one-time setup ----
            # POOL order matters: step broadcast + iota first (they gate the
            # guide -> ksc chain, which gates the first att tile).
            step_sb = const.tile([P, 1], F32)
            nc.gpsimd.dma_start(out=step_sb, in_=stp.to_broadcast((P, 1)))
            pos = setup.tile([P, N], F32)
            for h2 in range(2):
                nc.gpsimd.iota(
                    pos[:, ts(h2, N // 2)],
                    pattern=[[1, N // 2]],
                    base=1 + h2 * (N // 2),
                    channel_multiplier=0,
                    allow_small_or_imprecise_dtypes=True,
                )

            # Wq^T tiles: wq_sb[p, lt, a] = Wq[a, lt*128+p]
            wq_sb = const.tile([P, LT, A], F16)
            nc.sync.dma_start(out=wq_sb, in_=wqT.rearrange("(lt p) a -> p lt a", p=P))

            nstep = const.tile([P, 1], F32)
            nc.vector.tensor_scalar_mul(nstep, step_sb, -1.0)

            neg_tile = const.tile([P, N], F32)
            nc.vector.memset(neg_tile, NEG)

            ident = const.tile([P, P], F32)
            make_identity(nc, ident)
            identh = const.tile([P, P], F16)
            nc.vector.tensor_copy(identh, ident)

            # guide row, replicated across all 128 partitions:
            #   guide[n] = exp(-((n+1)/N - step)^2 / TEMP - 0.5*ln(A))
            # (the 1/sqrt(A) attention norm is folded into the bias).
            # Computed in halves so ksc (and the first att tile) starts early.
            gbias = const.tile([P, 1], F32)
            nc.vector.memset(gbias, -0.5 * math.log(A))
            z = setup.tile([P, N], F32)
            guide = setup.tile([P, N], F16)

            # Q^T[a, t] = sum_l Wq[a, l] * query[t, l].
            # qT is loaded in four t-chunks so the projection (and the first
            # att tiles) can start before the whole 4 MiB query arrives.
            # DMA arrival order on the sync ring: wq, qt0, kT, v0..v3, qt1-3;
            # masks go on the scalar ring so they never queue behind these.
            QCH = TQ // 4
            qt_in = setup.tile([P, LT, TQ], F16)
            qt = const.tile([P, TQ], F16)
            qT_r = qT.rearrange("(lt p) t -> p lt t", p=P)
            v_sb = const.tile([P, NT, E], F16)
            v_r = v.rearrange("(nt p) e -> p nt e", p=P)

            def load_qt_chunk(q):
                nc.sync.dma_start(
                    out=qt_in[:, :, ts(q, QCH)], in_=qT_r[:, :, ts(q, QCH)]
                )

            def project_qt_chunk(q):
                ps_qt = psO.tile([P, QCH], F32, tag="pso", name="ps_qt")
                for lt in range(LT):
                    nc.tensor.matmul(
                        ps_qt,
                        wq_sb[:, lt, :],
                        qt_in[:, lt, ts(q, QCH)],
                        start=(lt == 0),
                        stop=(lt == LT - 1),
                    )
                nc.scalar.copy(qt[:, ts(q, QCH)], ps_qt)

            load_qt_chunk(0)
            kt_sb = setup.tile([P, N], F16)
            nc.scalar.dma_start(out=kt_sb, in_=kT)
            for vh in range(2):
                nc.gpsimd.dma_start(
                    out=v_sb[:, ts(vh, NT // 2), :], in_=v_r[:, ts(vh, NT // 2), :]
                )
            for q in range(1, 4):
                load_qt_chunk(q)

            ksc = const.tile([P, N], F16)
            for h2 in range(2):
                hs = ts(h2, N // 2)
                nc.scalar.activation(
                    out=z[:, hs],
                    in_=pos[:, hs],
                    func=mybir.ActivationFunctionType.Square,
                    bias=nstep,
                    scale=1.0 / N,
                )
                nc.scalar.activation(
                    out=guide[:, hs],
                    in_=z[:, hs],
                    func=mybir.ActivationFunctionType.Exp,
                    scale=-1.0 / TEMP,
                    bias=gbias,
                )
                nc.vector.tensor_mul(ksc[:, hs], kt_sb[:, hs], guide[:, hs])

            # ---- main loop: software-pipelined over 128-row tiles of Tq ----
            # Stage A(ti): mask DMA, att matmuls, mask-predicate, exp+rowsum.
            # Stage B(ti): transposes, AV matmuls, normalize, store.
            # B(ti-1) is emitted after A(ti): the PE stream then interleaves
            # att(ti) with transpose/AV(ti-1), so the PE never sits waiting on
            # exp(ti) and the HAM clock stays unthrottled.
            H = N // 2
            stash = {}

            def stage_a(ti):
                mk = mpool.tile([P, N], U8, name="mk")
                nc.scalar.dma_start(out=mk, in_=msk[ts(ti, P), :])
                s = spool.tile([P, N], F16, name="s")
                rs2 = small.tile([P, 2], F32, name="rs2")
                for h in range(2):
                    att = psA.tile([P, H], F32, tag="att", name="att")
                    for j in range(H // 512):
                        nc.tensor.matmul(
                            att[:, ts(j, 512)],
                            qt[:, ts(ti, P)],
                            ksc[:, ts(h * 2 + j, 512)],
                            start=True,
                            stop=True,
                        )
                    # masked lanes -> -200 (exp underflows to 0)
                    nc.vector.copy_predicated(
                        out=att, mask=mk[:, ts(h, H)], data=neg_tile[:, ts(h, H)]
                    )
                    # s = exp(att) in f16; rs = f32 row-sum fused on ScalarE
                    nc.scalar.activation(
                        out=s[:, ts(h, H)],
                        in_=att,
                        func=mybir.ActivationFunctionType.Exp,
                        accum_out=rs2[:, h : h + 1],
                    )
                stash[ti] = (s, rs2)

            def stage_b(ti):
                s, rs2 = stash.pop(ti)
                rs = small.tile([P, 1], F32, name="rs")
                nc.vector.tensor_reduce(
                    out=rs, in_=rs2, axis=mybir.AxisListType.X, op=mybir.AluOpType.add
                )
                rc = small.tile([P, 1], F32, name="rc")
                nc.vector.reciprocal(rc, rs)

                # s^T: st[p, i, t] = s[t, i*128+p] via PE transpose.
                # (Both DMA xbar-transpose variants lose: an SBUF-source
                # transpose hard-faults the device, and the DRAM-bounce path
                # serializes the DMA rings and costs ~50us end-to-end.)
                st = stpool.tile([P, NT, P], F16, name="st")
                st_flat = st.rearrange("p i t -> p (i t)")
                for g in range(2):
                    ps_tr = psO.tile([P, 1024], F16, tag="pstr", name="ps_tr")
                    for j in range(8):
                        nc.tensor.transpose(
                            ps_tr[:, ts(j, P)],
                            s[:, ts(g * 8 + j, P)],
                            identh,
                        )
                    if g % 2 == 0:
                        nc.scalar.copy(st_flat[:, ts(g, 1024)], ps_tr)
                    else:
                        nc.vector.tensor_copy(st_flat[:, ts(g, 1024)], ps_tr)

                # out[t, e] = sum_n s[t, n] * V[n, e], accumulated over 16 n-tiles
                ot = psO.tile([P, E], F32, tag="pso", name="ot")
                for nt in range(NT):
                    nc.tensor.matmul(
                        ot,
                        st[:, nt, :],
                        v_sb[:, nt, :],
                        start=(nt == 0),
                        stop=(nt == NT - 1),
                    )

                # normalize on the narrow output tile and store
                ob = opool.tile([P, E], F32, name="ob")
                nc.vector.tensor_scalar_mul(ob, ot, rc)
                nc.sync.dma_start(out=out[ts(ti, P), :], in_=ob)

            # skewed emission; Q-projection chunks are interleaved just before
            # the first att tile that needs them (att(ti) reads qt chunk ti//2)
            project_qt_chunk(0)
            stage_a(0)
            for ti in range(1, TT):
                if ti % 2 == 0:
                    project_qt_chunk(ti // 2)
                stage_a(ti)
                stage_b(ti - 1)
            stage_b(TT - 1)

    nc.compile()
    return nc


def make_in_maps(query, K, V, Wq, step, mask):
    query = np.asarray(query, dtype=np.float32)
    K = np.asarray(K, dtype=np.float32)
    V = np.asarray(V, dtype=np.float32)
    Wq = np.asarray(Wq, dtype=np.float32)
    step = np.asarray(step, dtype=np.float32)
    mask = np.asarray(mask)
    if mask.dtype != np.uint8:
        mask = mask.astype(np.uint8)

    wqT = np.ascontiguousarray(Wq.T).astype(np.float16)
    stp = step.reshape(1, 1)
    in_maps = []
    for b in range(B):
        in_maps.append(
            {
                "qT": np.ascontiguousarray(query[b].T).astype(np.float16),
                "kT": np.ascontiguousarray(K[b].T).astype(np.float16),
                "v": np.ascontiguousarray(V[b]).astype(np.float16),
                "wqT": wqT,
                "stp": stp,
                "msk": mask[b],
            }
        )
    return in_maps


def kernel(query, K, V, Wq, step, mask):
    nc = build_nc()
    in_maps = make_in_maps(query, K, V, Wq, step, mask)
    res = run_bass_kernel_spmd(nc, in_maps, core_ids=list(range(B)))
    return np.stack([res.results[b]["out"] for b in range(B)], axis=0)


if __name__ == "__main__":
    rng = np.random.default_rng(0)
    inputs = {
        "query": rng.standard_normal((B, TQ, L), dtype=np.float32),
        "K": rng.standard_normal((B, N, A), dtype=np.float32),
        "V": rng.standard_normal((B, N, E), dtype=np.float32),
        "Wq": rng.standard_normal((A, L), dtype=np.float32) / math.sqrt(L),
        "step": rng.random((1,), dtype=np.float32),
        "mask": rng.integers(0, 2, size=(B, TQ, N)) > 0,
    }
    out = kernel(**inputs)
    print(out.shape, out.dtype)


# revision 18
# speedup vs baseline: 1.0389x; 1.0389x over previous
"""Trainium2 Bass kernel for batched cross-attention with gaussian guide mask.

Reference computation (per batch b):
  Q   = query @ Wq.T                      # [Tq, A]
  att = (Q @ K.T / sqrt(A)) * guide       # guide[n] = exp(-(step-(n+1)/N)^2/TEMP)
  att = where(mask, -inf, att)
  out = softmax(att, axis=-1) @ V         # [Tq, E]

Sharding: data-parallel over batch. Core b handles batch b (B == 8 == n_cores).

Device-side layout choices (host does layout-only prep: transposes/casts):
  qT  = query[b].T   [L, Tq]   - so Q^T comes out of PE with A on partitions
  kT  = K[b].T       [A, N]    - guide and 1/sqrt(A) folded in on device
  v   = V[b]         [N, E]    - natural; AV contraction tiles n on partitions
  wqT = Wq.T         [L, A]
  msk = mask[b] u8   [Tq, N]
Softmax is computed without max-subtraction (att values are O(5), exp is safe
in f32, and softmax is shift-invariant); masked lanes are set to -200 before
exp so exp underflows to 0 and the fused accumulate row-sum is exact.
Normalization is applied to the [128, 512] output tile instead of the
[128, 2048] score tile (linearity of the AV matmul).
"""

import math

import ml_dtypes
import numpy as np

import concourse.bass as bass
import concourse.mybir as mybir
import concourse.tile as tile
from concourse import bacc
from concourse.bass import ts
from concourse.bass_utils import run_bass_kernel_spmd
from concourse.masks import make_identity

B, TQ, N = 8, 1024, 2048
L, A, E = 1024, 128, 512
TEMP = 0.08
P = 128
LT = L // P    # 8 l-tiles (contraction tiles of the Q projection)
TT = TQ // P   # 8 t-tiles (rows of attention, 128 at a time)
NT = N // P    # 16 n-tiles (contraction tiles of the AV matmul)
NEG = -200.0   # masked logit value; exp(-200) underflows to exactly 0 in f32

import os

USE_DMA_TRANSPOSE = os.environ.get("KDMAT", "0") == "1"

F32 = mybir.dt.float32
F32R = mybir.dt.float32r
F16 = mybir.dt.float16
U8 = mybir.dt.uint8


def build_nc():
    nc = bacc.Bacc("TRN2", target_bir_lowering=False, debug=False, enable_asserts=False, num_devices=B)

    qT = nc.dram_tensor("qT", [4 * P, LT * 256], F16, kind="ExternalInput").ap()
    kT = nc.dram_tensor("kT", [A, N], F16, kind="ExternalInput").ap()
    v = nc.dram_tensor("v", [P, NT * E], F16, kind="ExternalInput").ap()
    wqT = nc.dram_tensor("wqT", [P, LT * A], F16, kind="ExternalInput").ap()
    stp = nc.dram_tensor("stp", [1, 1], F32, kind="ExternalInput").ap()
    msk = nc.dram_tensor("msk", [TQ, N], U8, kind="ExternalInput").ap()
    out = nc.dram_tensor("out", [TQ, E], F32, kind="ExternalOutput").ap()

    with tile.TileContext(nc) as tc:
        with (
            tc.tile_pool(name="const", bufs=1) as const,
            tc.tile_pool(name="setup", bufs=1) as setup,
            tc.tile_pool(name="mpool", bufs=3) as mpool,
            tc.tile_pool(name="spool", bufs=3) as spool,
            tc.tile_pool(name="stpool", bufs=3) as stpool,
            tc.tile_pool(name="opool", bufs=3) as opool,
            tc.tile_pool(name="small", bufs=6) as small,
            tc.tile_pool(
                name="psA", bufs=3 if USE_DMA_TRANSPOSE else 2, space="PSUM"
            ) as psA,
            tc.tile_pool(name="psO", bufs=2, space="PSUM") as psO,
        ):
            # ---- one-time setup ----
            # POOL order matters: step broadcast + iota first (they gate the
            # guide -> ksc chain, which gates the first att tile).
            step_sb = const.tile([P, 1], F32)
            nc.gpsimd.dma_start(out=step_sb, in_=stp.to_broadcast((P, 1)))
            pos = setup.tile([P, N], F32)
            for h2 in range(2):
                nc.gpsimd.iota(
                    pos[:, ts(h2, N // 2)],
                    pattern=[[1, N // 2]],
                    base=1 + h2 * (N // 2),
                    channel_multiplier=0,
                    allow_small_or_imprecise_dtypes=True,
                )

            # Wq^T tiles: wq_sb[p, lt, a] = Wq[a, lt*128+p]
            wq_sb = const.tile([P, LT, A], F16)
            nc.sync.dma_start(out=wq_sb, in_=wqT)

            nstep = const.tile([P, 1], F32)
            nc.vector.tensor_scalar_mul(nstep, step_sb, -1.0)

            neg_tile = const.tile([P, N], F32)
            nc.vector.memset(neg_tile, NEG)

            ident = const.tile([P, P], F32)
            make_identity(nc, ident)
            identh = const.tile([P, P], F16)
            nc.vector.tensor_copy(identh, ident)

            # guide row, replicated across all 128 partitions:
            #   guide[n] = exp(-((n+1)/N - step)^2 / TEMP - 0.5*ln(A))
            # (the 1/sqrt(A) attention norm is folded into the bias).
            # Computed in halves so ksc (and the first att tile) starts early.
            gbias = const.tile([P, 1], F32)
            nc.vector.memset(gbias, -0.5 * math.log(A))
            z = setup.tile([P, N], F32)
            guide = setup.tile([P, N], F16)

            # Q^T[a, t] = sum_l Wq[a, l] * query[t, l].
            # qT is loaded in four t-chunks so the projection (and the first
            # att tiles) can start before the whole 4 MiB query arrives.
            # DMA arrival order on the sync ring: wq, qt0, kT, v0..v3, qt1-3;
            # masks go on the scalar ring so they never queue behind these.
            QCH = TQ // 4
            qt_in = setup.tile([P, 4, LT, QCH], F16)
            qt = const.tile([P, TQ], F16)
            v_sb = const.tile([P, NT, E], F16)

            def load_qt_chunk(q):
                nc.sync.dma_start(out=qt_in[:, q, :, :], in_=qT[ts(q, P), :])

            def project_qt_chunk(q):
                ps_qt = psO.tile([P, QCH], F32, tag="pso", name="ps_qt")
                for lt in range(LT):
                    nc.tensor.matmul(
                        ps_qt,
                        wq_sb[:, lt, :],
                        qt_in[:, q, lt, :],
                        start=(lt == 0),
                        stop=(lt == LT - 1),
                    )
                nc.scalar.copy(qt[:, ts(q, QCH)], ps_qt)

            load_qt_chunk(0)
            kt_sb = setup.tile([P, N], F16)
            nc.scalar.dma_start(out=kt_sb, in_=kT)
            for vh in range(2):
                nc.gpsimd.dma_start(
                    out=v_sb[:, ts(vh, NT // 2), :],
                    in_=v[:, ts(vh, NT * E // 2)],
                )
            for q in range(1, 4):
                load_qt_chunk(q)

            ksc = const.tile([P, N], F16)
            for h2 in range(2):
                hs = ts(h2, N // 2)
                nc.scalar.activation(
                    out=z[:, hs],
                    in_=pos[:, hs],
                    func=mybir.ActivationFunctionType.Square,
                    bias=nstep,
                    scale=1.0 / N,
                )
                nc.scalar.activation(
                    out=guide[:, hs],
                    in_=z[:, hs],
                    func=mybir.ActivationFunctionType.Exp,
                    scale=-1.0 / TEMP,
                    bias=gbias,
                )
                nc.vector.tensor_mul(ksc[:, hs], kt_sb[:, hs], guide[:, hs])

            # ---- main loop: software-pipelined over 128-row tiles of Tq ----
            # Stage A(ti): mask DMA, att matmuls, mask-predicate, exp+rowsum.
            # Stage B(ti): transposes, AV matmuls, normalize, store.
            # B(ti-1) is emitted after A(ti): the PE stream then interleaves
            # att(ti) with transpose/AV(ti-1), so the PE never sits waiting on
            # exp(ti) and the HAM clock stays unthrottled.
            H = N // 2
            stash = {}

            def stage_a(ti):
                mk = mpool.tile([P, N], U8, name="mk")
                nc.scalar.dma_start(out=mk, in_=msk[ts(ti, P), :])
                s = spool.tile([P, N], F16, name="s")
                rs2 = small.tile([P, 2], F32, name="rs2")
                for h in range(2):
                    att = psA.tile([P, H], F32, tag="att", name="att")
                    for j in range(H // 512):
                        nc.tensor.matmul(
                            att[:, ts(j, 512)],
                            qt[:, ts(ti, P)],
                            ksc[:, ts(h * 2 + j, 512)],
                            start=True,
                            stop=True,
                        )
                    # masked lanes -> -200 (exp underflows to 0)
                    nc.vector.copy_predicated(
                        out=att, mask=mk[:, ts(h, H)], data=neg_tile[:, ts(h, H)]
                    )
                    # s = exp(att) in f16; rs = f32 row-sum fused on ScalarE
                    nc.scalar.activation(
                        out=s[:, ts(h, H)],
                        in_=att,
                        func=mybir.ActivationFunctionType.Exp,
                        accum_out=rs2[:, h : h + 1],
                    )
                stash[ti] = (s, rs2)

            def stage_b(ti):
                s, rs2 = stash.pop(ti)
                rs = small.tile([P, 1], F32, name="rs")
                nc.vector.tensor_reduce(
                    out=rs, in_=rs2, axis=mybir.AxisListType.X, op=mybir.AluOpType.add
                )
                rc = small.tile([P, 1], F32, name="rc")
                nc.vector.reciprocal(rc, rs)

                # s^T: st[p, i, t] = s[t, i*128+p] via PE transpose.
                # (Both DMA xbar-transpose variants lose: an SBUF-source
                # transpose hard-faults the device, and the DRAM-bounce path
                # serializes the DMA rings and costs ~50us end-to-end.)
                st = stpool.tile([P, NT, P], F16, name="st")
                st_flat = st.rearrange("p i t -> p (i t)")
                for g in range(2):
                    ps_tr = psO.tile([P, 1024], F16, tag="pstr", name="ps_tr")
                    for j in range(8):
                        nc.tensor.transpose(
                            ps_tr[:, ts(j, P)],
                            s[:, ts(g * 8 + j, P)],
                            identh,
                        )
                    if g % 2 == 0:
                        nc.scalar.copy(st_flat[:, ts(g, 1024)], ps_tr)
                    else:
                        nc.vector.tensor_copy(st_flat[:, ts(g, 1024)], ps_tr)

                # out[t, e] = sum_n s[t, n] * V[n, e], accumulated over 16 n-tiles
                ot = psO.tile([P, E], F32, tag="pso", name="ot")
                for nt in range(NT):
                    nc.tensor.matmul(
                        ot,
                        st[:, nt, :],
                        v_sb[:, nt, :],
                        start=(nt == 0),
                        stop=(nt == NT - 1),
                    )

                # normalize on the narrow output tile and store
                ob = opool.tile([P, E], F32, name="ob")
                nc.vector.tensor_scalar_mul(ob, ot, rc)
                nc.sync.dma_start(out=out[ts(ti, P), :], in_=ob)

            # skewed emission; Q-projection chunks are interleaved just before
            # the first att tile that needs them (att(ti) reads qt chunk ti//2)
            project_qt_chunk(0)
            stage_a(0)
            for ti in range(1, TT):
                if ti % 2 == 0:
                    project_qt_chunk(ti // 2)
                stage_a(ti)
                stage_b(ti - 1)
            stage_b(TT - 1)

    nc.compile()
    return nc


def make_in_maps(query, K, V, Wq, step, mask):
    query = np.asarray(query, dtype=np.float32)
    K = np.asarray(K, dtype=np.float32)
    V = np.asarray(V, dtype=np.float32)
    Wq = np.asarray(Wq, dtype=np.float32)
    step = np.asarray(step, dtype=np.float32)
    mask = np.asarray(mask)
    if mask.dtype != np.uint8:
        mask = mask.astype(np.uint8)

    # [p][lt][a] layout: contiguous 2 KiB per partition row
    wqT = np.ascontiguousarray(
        Wq.T.astype(np.float16).reshape(LT, P, A).transpose(1, 0, 2).reshape(P, LT * A)
    )
    stp = step.reshape(1, 1)
    in_maps = []
    for b in range(B):
        in_maps.append(
            {
                "qT": np.ascontiguousarray(
                    query[b]
                    .T.astype(np.float16)
                    .reshape(LT, P, 4, TQ // 4)
                    .transpose(2, 1, 0, 3)
                    .reshape(4 * P, LT * (TQ // 4))
                ),
                "kT": np.ascontiguousarray(K[b].T).astype(np.float16),
                "v": np.ascontiguousarray(
                    V[b]
                    .astype(np.float16)
                    .reshape(NT, P, E)
                    .transpose(1, 0, 2)
                    .reshape(P, NT * E)
                ),
                "wqT": wqT,
                "stp": stp,
                "msk": mask[b],
            }
        )
    return in_maps


def kernel(query, K, V, Wq, step, mask):
    nc = build_nc()
    in_maps = make_in_maps(query, K, V, Wq, step, mask)
    res = run_bass_kernel_spmd(nc, in_maps, core_ids=list(range(B)))
    return np.stack([res.results[b]["out"] for b in range(B)], axis=0)


if __name__ == "__main__":
    rng = np.random.default_rng(0)
    inputs = {
        "query": rng.standard_normal((B, TQ, L), dtype=np.float32),
        "K": rng.standard_normal((B, N, A), dtype=np.float32),
        "V": rng.standard_normal((B, N, E), dtype=np.float32),
        "Wq": rng.standard_normal((A, L), dtype=np.float32) / math.sqrt(L),
        "step": rng.random((1,), dtype=np.float32),
        "mask": rng.integers(0, 2, size=(B, TQ, N)) > 0,
    }
    out = kernel(**inputs)
    print(out.shape, out.dtype)
